# revision 49
# baseline (speedup 1.0000x reference)
"""Trainium2 Bass kernel for a 32-head causal attention layer.

Problem: B=1, S=2048, D=4096, 32 heads x 128 head-dim, fp32, llama-style
interleaved RoPE on q/k, KV-cache index_copy (identity for arange indexes),
additive mask + softmax, output projection.

Sharding (8 NeuronCores, tensor-parallel by heads):
  - core c owns heads [4c, 4c+4): wq/wk/wv output rows [512c, 512c+512)
  - per-core: QKV projections -> RoPE -> attention -> attn.T shard [512, 2048]
  - 4 chunked on-device AllGathers (one per 512-query block)
  - core c computes output column shard out[:, 512c:512c+512] = attn @ wo_c.T
  - host concatenates the 8 column shards (pure unshard, no arithmetic)

Pipelined schedule: causal attention for query block qb only needs K/V of
seq chunks <= qb, so the program interleaves
  QKV(0), attn(0)+AG(0), QKV(1), attn(1)+AG(1), P4(0a), QKV(2),
  attn(2)+AG(2), P4(1a), QKV(3), attn(3)+AG(3), P4(0b), P4(1b), P4(2), P4(3)
which starts the collectives ~4x earlier and keeps the tensor engine
continuously busy (holding its p-state near max clock).  P4(0)/P4(1) are
split in seq halves so ~65us of output-projection matmuls sit between the
last AllGather's launch and P4(3)'s first use of its data — the ring
flight varies 35-140us with fabric load and is hidden up to ~70us.
Collectives never overlap attention compute (an in-flight collective
stalls the exp stream via a shared-semaphore artifact), and each AG fires
once per block: the ring is latency-bound (~40us regardless of payload) so
splitting a collective never pays.  Tail P4 at-loads alternate sync/scalar
because one queue's descriptor-issue rate cannot feed 2-matmul chunks;
mid-pipeline at-loads ride gpsimd so a pending AllGather never delays the
QKV weight streams.

All matmul operands are bf16 (1 cycle/row on the PE, half the DMA bytes of
fp32r); PSUM accumulation stays fp32.  End-to-end max-abs error vs the fp32
reference is ~3e-3 of the output max (budget 2e-2).

RoPE trick: weight rows of wq/wk are permuted per head on the host so the
interleaved pairs (2j, 2j+1) become (j, j+64).  Scores are invariant under
a per-head orthogonal permutation applied to both q and k, and the rotation
then only needs partition-range [0:64]/[64:128] cross-multiplies, which map
to plain DVE tensor_tensor ops (no strided partition access).

The 1/sqrt(128) score scale is folded into the Exp activation's scale
operand.  Softmax runs over the partition (key) axis: scores are computed
transposed st[k, q] = K Q^T, summed with a ones-vector matmul, and
normalized after the PV matmul via a reciprocal + outer-product broadcast.
"""

import numpy as np

import concourse.bass as bass
import concourse.mybir as mybir
import concourse.tile as tile
from concourse import bacc
from concourse.bass_utils import run_bass_kernel_spmd

F32 = mybir.dt.float32
BF16 = mybir.dt.bfloat16

S = 2048
D = 4096
HD = 128
N_HEADS = 32
N_CORES = 8
HPC = N_HEADS // N_CORES          # heads per core = 4
FC = HPC * HD                     # features per core = 512
N_DC = D // 128                   # 32 contraction chunks
N_SC = S // 512                   # 4 seq chunks of 512
N_KC = S // 128                   # 16 key chunks of 128
SCALE = 1.0 / np.sqrt(HD)
NEG = -1e9


def _build_module(causal: bool):
    nc = bacc.Bacc(num_devices=N_CORES)

    xT = nc.dram_tensor("xT", [N_SC * D, 512], BF16, kind="ExternalInput")
    wqk_t = nc.dram_tensor("wqk_t", [D, 2 * FC], BF16, kind="ExternalInput")
    wv_t = nc.dram_tensor("wv_t", [D, FC], BF16, kind="ExternalInput")
    wo_t = nc.dram_tensor("wo_t", [D, FC], BF16, kind="ExternalInput")
    cosb = nc.dram_tensor("cosb", [128, S], F32, kind="ExternalInput")
    sinb = nc.dram_tensor("sinb", [128, S], F32, kind="ExternalInput")
    if causal:
        bmask = nc.dram_tensor("bmask", [128, 4 * 512], BF16, kind="ExternalInput")
    else:
        maskT = nc.dram_tensor("maskT", [S, S], F32, kind="ExternalInput")
    out_t = nc.dram_tensor("out", [S, FC], F32, kind="ExternalOutput")

    with tile.TileContext(nc) as tc:
        with tc.tile_pool(name="const", bufs=1) as constp, \
             tc.tile_pool(name="dram", bufs=1, space="DRAM") as dram, \
             tc.tile_pool(name="sb", bufs=1) as sb, \
             tc.tile_pool(name="ps", bufs=1, space="PSUM") as ps:
            cc_in = [dram.tile([FC, 512], BF16, name=f"cc_in{i}")
                     for i in range(N_SC)]
            cc_out = [dram.tile([D, 512], BF16, addr_space="Shared",
                                name=f"cc_out{i}") for i in range(N_SC)]

            ones_f = constp.tile([128, 1], F32, tag="ones_f")
            nc.vector.memset(ones_f[:], 1.0)
            ones_col = constp.tile([128, 1], BF16, tag="ones_col")
            nc.vector.tensor_copy(ones_col[:], ones_f[:])



            cos_sb = constp.tile([128, S], F32, tag="cos")
            sin_sb = constp.tile([128, S], F32, tag="sin")
            nc.gpsimd.dma_start(cos_sb[:], cosb[:])
            nc.gpsimd.dma_start(sin_sb[:], sinb[:])
            if causal:
                bm_sb = constp.tile([128, 4 * 512], BF16, tag="bm")
                nc.gpsimd.dma_start(bm_sb[:], bmask[:])

            # persistent q/k/v activation tiles (bf16)
            qt = [constp.tile([128, S], BF16, tag=f"qt{h}", name=f"qt{h}")
                  for h in range(HPC)]
            kt = [constp.tile([128, S], BF16, tag=f"kt{h}", name=f"kt{h}")
                  for h in range(HPC)]
            vt = [constp.tile([128, FC], BF16, tag=f"vt{b}", name=f"vt{b}")
                  for b in range(N_KC)]

            # resident wo tiles: on the gpsimd queue so they do not delay the
            # QKV weight/activation streams (sync+scalar queues)
            wo_sb = [constp.tile([128, FC], BF16, tag=f"wo{dc}", name=f"wo{dc}")
                     for dc in range(N_DC)]
            for dc in range(N_DC):
                nc.gpsimd.dma_start(wo_sb[dc][:], wo_t[dc * 128:(dc + 1) * 128, :])

            def rope_evict(acc, dst, sc):
                s0 = sc * 512
                cs = cos_sb[:, s0:s0 + 512]
                sn = sin_sb[:, s0:s0 + 512]
                t14 = sb.tile([128, 512], F32, tag="t1", bufs=2)
                t2 = sb.tile([128, 512], F32, tag="t2", bufs=2)
                t3 = sb.tile([128, 512], F32, tag="t3", bufs=2)
                mul = mybir.AluOpType.mult
                # acc reads first (3 ops, first full-width: cos_sb holds the
                # cos table duplicated in both halves) so the PSUM bank frees
                # ASAP for the next sweep's matmuls
                nc.vector.tensor_tensor(out=t14[:], in0=acc[:], in1=cs[:],
                                        op=mul)
                nc.vector.tensor_tensor(out=t2[0:64, :], in0=acc[64:128, :],
                                        in1=sn[0:64, :], op=mul)
                nc.vector.tensor_tensor(out=t3[64:128, :], in0=acc[0:64, :],
                                        in1=sn[64:128, :], op=mul)
                nc.vector.tensor_tensor(out=dst[0:64, s0:s0 + 512],
                                        in0=t14[0:64, :], in1=t2[0:64, :],
                                        op=mybir.AluOpType.subtract)
                nc.vector.tensor_tensor(out=dst[64:128, s0:s0 + 512],
                                        in0=t14[64:128, :], in1=t3[64:128, :],
                                        op=mybir.AluOpType.add)

            def qkv_phase(sc):
                s0 = sc * 512
                # q sweep (4 PSUM accumulators), then k sweep, then v sweep.
                xts = []
                for which in ("q", "k"):
                    off = 0 if which == "q" else FC
                    accs = [ps.tile([128, 512], F32, tag="acc", bufs=4,
                                    name=f"{which}ps{sc}_{h}") for h in range(HPC)]
                    for dc in range(N_DC):
                        d0 = dc * 128
                        if which == "q":
                            xt_sb = sb.tile([128, 512], BF16, tag="xt", bufs=40,
                                            name=f"xt{sc}_{dc}")
                            nc.sync.dma_start(
                                xt_sb[:], xT[sc * D + d0:sc * D + d0 + 128, :])
                            xts.append(xt_sb)
                        w_sb = sb.tile([128, FC], BF16, tag="wqk", bufs=12,
                                       name=f"w{which}{sc}_{dc}")
                        if which == "q":
                            weng = nc.scalar  # sync is busy with the xt loads
                        else:
                            weng = nc.scalar if dc % 2 == 0 else nc.sync
                        weng.dma_start(w_sb[:], wqk_t[d0:d0 + 128, off:off + FC])
                        for h in range(HPC):
                            f0 = h * 128
                            nc.tensor.matmul(accs[h][:], w_sb[:, f0:f0 + 128],
                                             xts[dc][:], start=(dc == 0),
                                             stop=(dc == N_DC - 1))
                    dst = qt if which == "q" else kt
                    for h in range(HPC):
                        rope_evict(accs[h], dst[h], sc)

                v_ps = [ps.tile([128, 512], F32, tag="acc", bufs=4,
                                name=f"vps{sc}_{i}") for i in range(4)]
                for dc in range(N_DC):
                    d0 = dc * 128
                    wv_sb = sb.tile([128, FC], BF16, tag="wv", bufs=12,
                                    name=f"wv{sc}_{dc}")
                    veng = nc.scalar if dc % 2 == 0 else nc.sync
                    veng.dma_start(wv_sb[:], wv_t[d0:d0 + 128, :])
                    for sbk in range(4):
                        nc.tensor.matmul(v_ps[sbk][:],
                                         xts[dc][:, sbk * 128:(sbk + 1) * 128],
                                         wv_sb[:], start=(dc == 0),
                                         stop=(dc == N_DC - 1))
                for sbk in range(4):
                    nc.vector.tensor_copy(vt[sc * 4 + sbk][:], v_ps[sbk][:])

            def attn_partial(qb, klo, khi):
                """Attention for query block qb over key chunks [klo, khi],
                normalizing and AllGathering the result."""
                q0 = qb * 512
                for h in range(HPC):
                    out_ps = ps.tile([128, 512], F32, tag="acc", bufs=4,
                                     name=f"o{qb}_{h}_{klo}")
                    sums_ps = ps.tile([1, 512], F32, tag="sums", bufs=2,
                                      name=f"s{qb}_{h}_{klo}")

                    def issue_st(kc):
                        k0 = kc * 128
                        # diagonal chunk j masks all query columns < j*128:
                        # narrow st/exp/PV/sums to the valid range (the
                        # skipped columns already hold contributions from
                        # earlier full-width chunks)
                        j = kc - 4 * qb if causal else -1
                        qlo = j * 128 if 1 <= j <= 3 else 0
                        st_ps = ps.tile([128, 512], F32, tag="st", bufs=2,
                                        name=f"st{qb}_{h}_{kc}")
                        nc.tensor.matmul(st_ps[:, qlo:512],
                                         kt[h][:, k0:k0 + 128],
                                         qt[h][:, q0 + qlo:q0 + 512],
                                         start=True, stop=True)
                        e_sb = sb.tile([128, 512], BF16, tag="e", bufs=8,
                                       name=f"e{qb}_{h}_{kc}")
                        if causal:
                            nc.scalar.activation(
                                e_sb[:, qlo:512], st_ps[:, qlo:512],
                                mybir.ActivationFunctionType.Exp,
                                scale=float(SCALE))
                            if 0 <= j <= 3:
                                nc.vector.tensor_tensor(
                                    out=e_sb[:, qlo:512],
                                    in0=e_sb[:, qlo:512],
                                    in1=bm_sb[:, j * 512 + qlo:(j + 1) * 512],
                                    op=mybir.AluOpType.mult)
                        else:
                            mt_sb = sb.tile([128, 512], F32, tag="mt", bufs=2,
                                            name=f"mt{qb}_{h}_{kc}")
                            nc.sync.dma_start(
                                mt_sb[:], maskT[k0:k0 + 128, q0:q0 + 512])
                            nc.vector.tensor_tensor(
                                out=st_ps[:], in0=st_ps[:], in1=mt_sb[:],
                                op=mybir.AluOpType.add)
                            nc.scalar.activation(
                                e_sb[:], st_ps[:],
                                mybir.ActivationFunctionType.Exp,
                                scale=float(SCALE))
                        return e_sb, qlo

                    e_cur, ql_cur = issue_st(klo)
                    for kc in range(klo, khi + 1):
                        e_next, ql_next = (issue_st(kc + 1) if kc < khi
                                           else (None, 0))
                        nc.tensor.matmul(out_ps[:, ql_cur:512],
                                         vt[kc][:, h * 128:(h + 1) * 128],
                                         e_cur[:, ql_cur:512],
                                         start=(kc == klo), stop=(kc == khi),
                                         skip_group_check=True)
                        nc.tensor.matmul(sums_ps[:, ql_cur:512], ones_col[:],
                                         e_cur[:, ql_cur:512],
                                         start=(kc == klo), stop=(kc == khi),
                                         skip_group_check=True)
                        e_cur, ql_cur = e_next, ql_next
                    r_sb = sb.tile([1, 512], F32, tag="r", bufs=2,
                                   name=f"r{qb}_{h}")
                    nc.vector.reciprocal(r_sb[:], sums_ps[:])
                    rb_sb = sb.tile([128, 512], F32, tag="rbs", bufs=2,
                                    name=f"rbs{qb}_{h}")
                    nc.gpsimd.partition_broadcast(rb_sb[:], r_sb[:])
                    a_sb = sb.tile([128, 512], BF16, tag="a", bufs=2,
                                   name=f"a{qb}_{h}")
                    nc.vector.tensor_tensor(out=a_sb[:], in0=out_ps[:],
                                            in1=rb_sb[:],
                                            op=mybir.AluOpType.mult)
                    nc.sync.dma_start(
                        cc_in[qb][h * 128:(h + 1) * 128, :], a_sb[:])
                # one AllGather per block, fired AFTER the last head: a
                # collective in flight during attention stalls the exp stream
                # (shared-semaphore artifact), and the ring is latency-bound
                # (~40us regardless of size) so splitting it never pays
                nc.gpsimd.collective_compute(
                    "AllGather",
                    mybir.AluOpType.bypass,
                    replica_groups=[list(range(N_CORES))],
                    ins=[cc_in[qb][:].opt()],
                    outs=[cc_out[qb][:].opt()],
                )

            def p4_run(sc, o_ps, sbks, suffix="", tail=False):
                # mid-pipeline at-loads go on the gpsimd queue so a pending
                # AllGather never blocks the QKV weight streams; tail at-loads
                # alternate sync/scalar (idle there) because one queue's
                # descriptor-issue rate cannot feed 2-matmul chunks
                for dc in range(N_DC):
                    at_sb = sb.tile([128, 512], BF16, tag="at", bufs=8,
                                    name=f"at{sc}_{dc}{suffix}")
                    if tail:
                        aeng = nc.sync if dc % 2 == 0 else nc.scalar
                    else:
                        aeng = nc.gpsimd
                    aeng.dma_start(at_sb[:],
                                   cc_out[sc][dc * 128:dc * 128 + 128, :])
                    for i, sbk in enumerate(sbks):
                        nc.tensor.matmul(o_ps[i][:],
                                         at_sb[:, sbk * 128:(sbk + 1) * 128],
                                         wo_sb[dc][:], start=(dc == 0),
                                         stop=(dc == N_DC - 1))

            def p4_evict(sc, o_ps, sbks):
                s0 = sc * 512
                for i, sbk in enumerate(sbks):
                    o_sb = sb.tile([128, FC], F32, tag="osb", bufs=2,
                                   name=f"osb{sc}_{sbk}")
                    nc.vector.tensor_copy(o_sb[:], o_ps[i][:])
                    r0 = s0 + sbk * 128
                    nc.scalar.dma_start(out_t[r0:r0 + 128, :], o_sb[:])

            def p4_phase(sc):
                o_ps = [ps.tile([128, 512], F32, tag="acc", bufs=4,
                                name=f"ops{sc}_{i}") for i in range(4)]
                p4_run(sc, o_ps, range(4))
                p4_evict(sc, o_ps, range(4))

            # pipelined schedule; P4(2) is placed after attn(3) so it hides
            # the last AllGather's flight before P4(3) consumes it
            last = N_SC - 1
            kmax_of = (lambda qb: 4 * qb + 3) if causal else (lambda qb: N_KC - 1)
            if not causal:
                # a general mask needs K/V of every block before any
                # attention, so QKV cannot be pipelined with it
                for sc in range(N_SC):
                    qkv_phase(sc)
            for sc in range(N_SC):
                if causal:
                    qkv_phase(sc)
                attn_partial(sc, 0, kmax_of(sc))
                if sc == 1:
                    # only half of P4(0)/P4(1) run mid-pipeline; the other
                    # halves run after attn(3) so ~65us of tensor work covers
                    # the last AllGather's ring flight (which varies 35-140us
                    # with fabric load) before P4(3) needs its data
                    o0a = [ps.tile([128, 512], F32, tag="acc", bufs=4,
                                   name=f"ops0a_{i}") for i in range(2)]
                    p4_run(0, o0a, (0, 1), suffix="a")
                    p4_evict(0, o0a, (0, 1))
                elif sc == 2:
                    o1a = [ps.tile([128, 512], F32, tag="acc", bufs=4,
                                   name=f"ops1a_{i}") for i in range(2)]
                    p4_run(1, o1a, (0, 1), suffix="a")
                    p4_evict(1, o1a, (0, 1))
            o0b = [ps.tile([128, 512], F32, tag="sums", bufs=2,
                           name=f"ops0b_{i}") for i in range(2)]
            p4_run(0, o0b, (2, 3), suffix="b", tail=True)
            p4_evict(0, o0b, (2, 3))
            o1b = [ps.tile([128, 512], F32, tag="st", bufs=2,
                           name=f"ops1b_{i}") for i in range(2)]
            p4_run(1, o1b, (2, 3), suffix="b", tail=True)
            p4_evict(1, o1b, (2, 3))
            o2t = [ps.tile([128, 512], F32, tag="acc", bufs=4,
                           name=f"ops{last-1}_{i}") for i in range(4)]
            p4_run(last - 1, o2t, range(4), tail=True)
            p4_evict(last - 1, o2t, range(4))
            o3 = [ps.tile([128, 512], F32, tag=t, bufs=2,
                          name=f"ops{last}_{i}")
                  for i, t in enumerate(("st", "st", "sums", "sums"))]
            p4_run(last, o3, range(4), tail=True)
            p4_evict(last, o3, range(4))

    nc.finalize()
    return nc


_MODULE_CACHE: dict = {}


def _get_module(causal: bool):
    if causal not in _MODULE_CACHE:
        _MODULE_CACHE[causal] = _build_module(causal)
    return _MODULE_CACHE[causal]


def _rope_perm() -> np.ndarray:
    """Per-head permutation: interleaved pairs (2j, 2j+1) -> (j, j+64)."""
    p = np.empty(HD, dtype=np.int64)
    p[0:64] = np.arange(0, HD, 2)
    p[64:128] = np.arange(1, HD, 2)
    full = np.concatenate([h * HD + p for h in range(HPC)])
    return full


def _canonical_causal_mask() -> np.ndarray:
    m = np.where(np.tril(np.ones((S, S), dtype=bool)), np.float32(0.0),
                 np.float32(NEG))
    return m.astype(np.float32)


def _numpy_fallback(x, freqs_cos, freqs_sin, mask, input_indexes, cache_k,
                    cache_v, wq, wk, wv, wo):
    """Exact reference reimplementation (host, fp32). Only used for inputs
    the device kernel does not model (non-arange cache indexes)."""
    B = x.shape[0]
    xf = x.astype(np.float32)

    def rope(t):
        tr = t[..., 0::2]
        ti = t[..., 1::2]
        c = freqs_cos[None, :, None, :]
        s = freqs_sin[None, :, None, :]
        outr = tr * c - ti * s
        outi = tr * s + ti * c
        return np.stack([outr, outi], axis=-1).reshape(t.shape)

    xq = (xf @ wq.T).reshape(B, S, N_HEADS, HD)
    xk = (xf @ wk.T).reshape(B, S, N_HEADS, HD)
    xv = (xf @ wv.T).reshape(B, S, N_HEADS, HD)
    xq = rope(xq)
    xk = rope(xk)
    keys = np.array(cache_k)
    vals = np.array(cache_v)
    keys[:, input_indexes] = xk
    vals[:, input_indexes] = xv
    scores = np.einsum("bqhd,bkhd->bhqk", xq, keys) / np.sqrt(HD)
    scores = scores + mask
    scores = scores - scores.max(axis=-1, keepdims=True)
    e = np.exp(scores)
    probs = e / e.sum(axis=-1, keepdims=True)
    out = np.einsum("bhqk,bkhd->bqhd", probs, vals)
    return (out.reshape(B, S, N_HEADS * HD) @ wo.T).astype(np.float32)


def _bf16(a: np.ndarray):
    import ml_dtypes
    return np.ascontiguousarray(a.astype(ml_dtypes.bfloat16))


def _prepare_in_maps(x, freqs_cos, freqs_sin, mask, wq, wk, wv, wo, causal):
    x2 = np.ascontiguousarray(x.reshape(S, D), dtype=np.float32)
    xTf = x2.T  # [D, S]
    xT = _bf16(np.concatenate(
        [xTf[:, sc * 512:(sc + 1) * 512] for sc in range(N_SC)], axis=0))

    cosb = np.empty((128, S), dtype=np.float32)
    sinb = np.empty((128, S), dtype=np.float32)
    fc = np.asarray(freqs_cos, dtype=np.float32).T  # [64, S]
    fs = np.asarray(freqs_sin, dtype=np.float32).T
    cosb[0:64] = fc
    cosb[64:128] = fc
    sinb[0:64] = fs
    sinb[64:128] = fs

    perm = _rope_perm()

    if causal:
        kl = np.arange(128, dtype=np.int64)[:, None]
        ql = np.arange(512, dtype=np.int64)[None, :]
        bmask = np.concatenate(
            [(kl <= ql - 128 * j).astype(np.float32) for j in range(4)], axis=1)
        bmask = _bf16(bmask)
    else:
        maskT = np.ascontiguousarray(
            (np.asarray(mask, dtype=np.float32)[0, 0].T) / np.float32(SCALE))

    in_maps = []
    for c in range(N_CORES):
        r0 = c * FC
        wq_c = np.asarray(wq[r0:r0 + FC], dtype=np.float32)[perm]
        wk_c = np.asarray(wk[r0:r0 + FC], dtype=np.float32)[perm]
        wqk_c = np.concatenate([wq_c, wk_c], axis=0)      # [1024, D]
        wv_c = np.asarray(wv[r0:r0 + FC], dtype=np.float32)
        wo_c = np.asarray(wo[r0:r0 + FC], dtype=np.float32)
        m = {
            "xT": xT,
            "wqk_t": _bf16(wqk_c.T),
            "wv_t": _bf16(wv_c.T),
            "wo_t": _bf16(wo_c.T),
            "cosb": cosb,
            "sinb": sinb,
        }
        if causal:
            m["bmask"] = bmask
        else:
            m["maskT"] = maskT
        in_maps.append(m)
    return in_maps


def _run(inputs: dict, trace: bool = False):
    x = np.asarray(inputs["x"])
    freqs_cos = np.asarray(inputs["freqs_cos"])
    freqs_sin = np.asarray(inputs["freqs_sin"])
    mask = np.asarray(inputs["mask"], dtype=np.float32)
    input_indexes = np.asarray(inputs["input_indexes"])
    wq = np.asarray(inputs["wq"])
    wk = np.asarray(inputs["wk"])
    wv = np.asarray(inputs["wv"])
    wo = np.asarray(inputs["wo"])

    if not np.array_equal(input_indexes.astype(np.int64), np.arange(S)):
        out = _numpy_fallback(x, freqs_cos, freqs_sin, mask, input_indexes,
                              inputs["cache_k"], inputs["cache_v"], wq, wk, wv, wo)
        return out, None

    causal = np.array_equal(mask[0, 0], _canonical_causal_mask())
    nc = _get_module(causal)
    in_maps = _prepare_in_maps(x, freqs_cos, freqs_sin, mask, wq, wk, wv, wo,
                               causal)
    res = run_bass_kernel_spmd(nc, in_maps, core_ids=list(range(N_CORES)),
                               trace=trace)
    out = np.concatenate([res.results[c]["out"] for c in range(N_CORES)],
                         axis=1)
    return out.reshape(1, S, D).astype(np.float32), res


def kernel(**inputs) -> np.ndarray:
    out, _ = _run(inputs, trace=False)
    return out


# revision 50
# speedup vs baseline: 1.0468x; 1.0468x over previous
"""Trainium2 Bass kernel for a 32-head causal attention layer.

Problem: B=1, S=2048, D=4096, 32 heads x 128 head-dim, fp32, llama-style
interleaved RoPE on q/k, KV-cache index_copy (identity for arange indexes),
additive mask + softmax, output projection.

Sharding (8 NeuronCores, tensor-parallel by heads):
  - core c owns heads [4c, 4c+4): wq/wk/wv output rows [512c, 512c+512)
  - per-core: QKV projections -> RoPE -> attention -> attn.T shard [512, 2048]
  - 4 chunked on-device AllGathers (one per 512-query block)
  - core c computes output column shard out[:, 512c:512c+512] = attn @ wo_c.T
  - host concatenates the 8 column shards (pure unshard, no arithmetic)

Pipelined schedule: causal attention for query block qb only needs K/V of
seq chunks <= qb, so the program interleaves
  QKV(0), attn(0)+AG(0), QKV(1), attn(1)+AG(1), P4(0a), QKV(2),
  attn(2)+AG(2), P4(1a), QKV(3), attn(3)+AG(3), P4(0b), P4(1b), P4(2), P4(3)
which starts the collectives ~4x earlier and keeps the tensor engine
continuously busy (holding its p-state near max clock).  P4(0)/P4(1) are
split in seq halves so ~65us of output-projection matmuls sit between the
last AllGather's launch and P4(3)'s first use of its data — the ring
flight varies 35-140us with fabric load and is hidden up to ~70us.
Collectives never overlap attention compute (an in-flight collective
stalls the exp stream via a shared-semaphore artifact), and each AG fires
once per block: the ring is latency-bound (~40us regardless of payload) so
splitting a collective never pays.  Tail P4 at-loads alternate sync/scalar
because one queue's descriptor-issue rate cannot feed 2-matmul chunks;
mid-pipeline at-loads ride gpsimd so a pending AllGather never delays the
QKV weight streams.

All matmul operands are bf16 (1 cycle/row on the PE, half the DMA bytes of
fp32r); PSUM accumulation stays fp32.  End-to-end max-abs error vs the fp32
reference is ~3e-3 of the output max (budget 2e-2).

RoPE trick: weight rows of wq/wk are permuted per head on the host so the
interleaved pairs (2j, 2j+1) become (j, j+64).  Scores are invariant under
a per-head orthogonal permutation applied to both q and k, and the rotation
then only needs partition-range [0:64]/[64:128] cross-multiplies, which map
to plain DVE tensor_tensor ops (no strided partition access).

The 1/sqrt(128) score scale is folded into the Exp activation's scale
operand.  Softmax runs over the partition (key) axis: scores are computed
transposed st[k, q] = K Q^T, summed with a ones-vector matmul, and
normalized after the PV matmul via a reciprocal + outer-product broadcast.
"""

import numpy as np

import concourse.bass as bass
import concourse.mybir as mybir
import concourse.tile as tile
from concourse import bacc
from concourse.bass_utils import run_bass_kernel_spmd

F32 = mybir.dt.float32
BF16 = mybir.dt.bfloat16

S = 2048
D = 4096
HD = 128
N_HEADS = 32
N_CORES = 8
HPC = N_HEADS // N_CORES          # heads per core = 4
FC = HPC * HD                     # features per core = 512
N_DC = D // 128                   # 32 contraction chunks
N_SC = S // 512                   # 4 seq chunks of 512
N_KC = S // 128                   # 16 key chunks of 128
SCALE = 1.0 / np.sqrt(HD)
NEG = -1e9


def _build_module(causal: bool):
    nc = bacc.Bacc(num_devices=N_CORES)

    xT = nc.dram_tensor("xT", [N_SC * D, 512], BF16, kind="ExternalInput")
    wqk_t = nc.dram_tensor("wqk_t", [D, 2 * FC], BF16, kind="ExternalInput")
    wv_t = nc.dram_tensor("wv_t", [D, FC], BF16, kind="ExternalInput")
    wo_t = nc.dram_tensor("wo_t", [D, FC], BF16, kind="ExternalInput")
    cosb = nc.dram_tensor("cosb", [128, S], F32, kind="ExternalInput")
    sinb = nc.dram_tensor("sinb", [128, S], F32, kind="ExternalInput")
    if causal:
        bmask = nc.dram_tensor("bmask", [128, 4 * 512], BF16, kind="ExternalInput")
    else:
        maskT = nc.dram_tensor("maskT", [S, S], F32, kind="ExternalInput")
    out_t = nc.dram_tensor("out", [S, FC], F32, kind="ExternalOutput")

    with tile.TileContext(nc) as tc:
        with tc.tile_pool(name="const", bufs=1) as constp, \
             tc.tile_pool(name="dram", bufs=1, space="DRAM") as dram, \
             tc.tile_pool(name="sb", bufs=1) as sb, \
             tc.tile_pool(name="ps", bufs=1, space="PSUM") as ps:
            cc_in = [dram.tile([FC, 512], BF16, name=f"cc_in{i}")
                     for i in range(N_SC)]
            cc_out = [dram.tile([D, 512], BF16, addr_space="Shared",
                                name=f"cc_out{i}") for i in range(N_SC)]

            ones_f = constp.tile([128, 1], F32, tag="ones_f")
            nc.vector.memset(ones_f[:], 1.0)
            ones_col = constp.tile([128, 1], BF16, tag="ones_col")
            nc.vector.tensor_copy(ones_col[:], ones_f[:])



            cos_sb = constp.tile([128, S], F32, tag="cos")
            sin_sb = constp.tile([128, S], F32, tag="sin")
            nc.gpsimd.dma_start(cos_sb[:], cosb[:])
            nc.gpsimd.dma_start(sin_sb[:], sinb[:])
            if causal:
                bm_sb = constp.tile([128, 4 * 512], BF16, tag="bm")
                nc.gpsimd.dma_start(bm_sb[:], bmask[:])

            # persistent q/k/v activation tiles (bf16)
            qt = [constp.tile([128, S], BF16, tag=f"qt{h}", name=f"qt{h}")
                  for h in range(HPC)]
            kt = [constp.tile([128, S], BF16, tag=f"kt{h}", name=f"kt{h}")
                  for h in range(HPC)]
            vt = [constp.tile([128, FC], BF16, tag=f"vt{b}", name=f"vt{b}")
                  for b in range(N_KC)]

            # resident wo tiles: on the gpsimd queue so they do not delay the
            # QKV weight/activation streams (sync+scalar queues)
            wo_sb = [constp.tile([128, FC], BF16, tag=f"wo{dc}", name=f"wo{dc}")
                     for dc in range(N_DC)]
            for dc in range(N_DC):
                nc.gpsimd.dma_start(wo_sb[dc][:], wo_t[dc * 128:(dc + 1) * 128, :])

            def rope_evict(acc, dst, sc):
                s0 = sc * 512
                cs = cos_sb[:, s0:s0 + 512]
                sn = sin_sb[:, s0:s0 + 512]
                t14 = sb.tile([128, 512], F32, tag="t1", bufs=2)
                t2 = sb.tile([128, 512], F32, tag="t2", bufs=2)
                t3 = sb.tile([128, 512], F32, tag="t3", bufs=2)
                mul = mybir.AluOpType.mult
                # acc reads first (3 ops, first full-width: cos_sb holds the
                # cos table duplicated in both halves) so the PSUM bank frees
                # ASAP for the next sweep's matmuls
                nc.vector.tensor_tensor(out=t14[:], in0=acc[:], in1=cs[:],
                                        op=mul)
                nc.vector.tensor_tensor(out=t2[0:64, :], in0=acc[64:128, :],
                                        in1=sn[0:64, :], op=mul)
                nc.vector.tensor_tensor(out=t3[64:128, :], in0=acc[0:64, :],
                                        in1=sn[64:128, :], op=mul)
                nc.vector.tensor_tensor(out=dst[0:64, s0:s0 + 512],
                                        in0=t14[0:64, :], in1=t2[0:64, :],
                                        op=mybir.AluOpType.subtract)
                nc.vector.tensor_tensor(out=dst[64:128, s0:s0 + 512],
                                        in0=t14[64:128, :], in1=t3[64:128, :],
                                        op=mybir.AluOpType.add)

            def qkv_phase(sc):
                s0 = sc * 512
                # q sweep (4 PSUM accumulators), then k sweep, then v sweep.
                xts = []
                for which in ("q", "k"):
                    off = 0 if which == "q" else FC
                    accs = [ps.tile([128, 512], F32, tag="acc", bufs=4,
                                    name=f"{which}ps{sc}_{h}") for h in range(HPC)]
                    for dc in range(N_DC):
                        d0 = dc * 128
                        if which == "q":
                            xt_sb = sb.tile([128, 512], BF16, tag="xt", bufs=40,
                                            name=f"xt{sc}_{dc}")
                            nc.sync.dma_start(
                                xt_sb[:], xT[sc * D + d0:sc * D + d0 + 128, :])
                            xts.append(xt_sb)
                        w_sb = sb.tile([128, FC], BF16, tag="wqk", bufs=12,
                                       name=f"w{which}{sc}_{dc}")
                        if which == "q":
                            weng = nc.scalar  # sync is busy with the xt loads
                        else:
                            weng = nc.scalar if dc % 2 == 0 else nc.sync
                        weng.dma_start(w_sb[:], wqk_t[d0:d0 + 128, off:off + FC])
                        for h in range(HPC):
                            f0 = h * 128
                            nc.tensor.matmul(accs[h][:], w_sb[:, f0:f0 + 128],
                                             xts[dc][:], start=(dc == 0),
                                             stop=(dc == N_DC - 1))
                    dst = qt if which == "q" else kt
                    for h in range(HPC):
                        rope_evict(accs[h], dst[h], sc)

                v_ps = [ps.tile([128, 512], F32, tag="acc", bufs=4,
                                name=f"vps{sc}_{i}") for i in range(4)]
                for dc in range(N_DC):
                    d0 = dc * 128
                    wv_sb = sb.tile([128, FC], BF16, tag="wv", bufs=12,
                                    name=f"wv{sc}_{dc}")
                    veng = nc.scalar if dc % 2 == 0 else nc.sync
                    veng.dma_start(wv_sb[:], wv_t[d0:d0 + 128, :])
                    for sbk in range(4):
                        nc.tensor.matmul(v_ps[sbk][:],
                                         xts[dc][:, sbk * 128:(sbk + 1) * 128],
                                         wv_sb[:], start=(dc == 0),
                                         stop=(dc == N_DC - 1))
                for sbk in range(4):
                    nc.vector.tensor_copy(vt[sc * 4 + sbk][:], v_ps[sbk][:])

            def attn_partial(qb, klo, khi):
                """Attention for query block qb over key chunks [klo, khi],
                normalizing and AllGathering the result."""
                q0 = qb * 512
                for h in range(HPC):
                    out_ps = ps.tile([128, 512], F32, tag="acc", bufs=4,
                                     name=f"o{qb}_{h}_{klo}")
                    sums_ps = ps.tile([1, 512], F32, tag="sums", bufs=2,
                                      name=f"s{qb}_{h}_{klo}")

                    def issue_st(kc):
                        k0 = kc * 128
                        # diagonal chunk j masks query columns < j*128 and
                        # could be narrowed, but per-matmul fixed overhead
                        # (~160ns) eats the saved rows: a 384-wide matmul
                        # measures 300ns vs 293ns for 512-wide, so keep full
                        # width
                        j = kc - 4 * qb if causal else -1
                        qlo = 0
                        st_ps = ps.tile([128, 512], F32, tag="st", bufs=2,
                                        name=f"st{qb}_{h}_{kc}")
                        nc.tensor.matmul(st_ps[:, qlo:512],
                                         kt[h][:, k0:k0 + 128],
                                         qt[h][:, q0 + qlo:q0 + 512],
                                         start=True, stop=True)
                        e_sb = sb.tile([128, 512], BF16, tag="e", bufs=8,
                                       name=f"e{qb}_{h}_{kc}")
                        if causal:
                            nc.scalar.activation(
                                e_sb[:, qlo:512], st_ps[:, qlo:512],
                                mybir.ActivationFunctionType.Exp,
                                scale=float(SCALE))
                            if 0 <= j <= 3:
                                nc.vector.tensor_tensor(
                                    out=e_sb[:, qlo:512],
                                    in0=e_sb[:, qlo:512],
                                    in1=bm_sb[:, j * 512 + qlo:(j + 1) * 512],
                                    op=mybir.AluOpType.mult)
                        else:
                            mt_sb = sb.tile([128, 512], F32, tag="mt", bufs=2,
                                            name=f"mt{qb}_{h}_{kc}")
                            nc.sync.dma_start(
                                mt_sb[:], maskT[k0:k0 + 128, q0:q0 + 512])
                            nc.vector.tensor_tensor(
                                out=st_ps[:], in0=st_ps[:], in1=mt_sb[:],
                                op=mybir.AluOpType.add)
                            nc.scalar.activation(
                                e_sb[:], st_ps[:],
                                mybir.ActivationFunctionType.Exp,
                                scale=float(SCALE))
                        return e_sb, qlo

                    e_cur, ql_cur = issue_st(klo)
                    for kc in range(klo, khi + 1):
                        e_next, ql_next = (issue_st(kc + 1) if kc < khi
                                           else (None, 0))
                        nc.tensor.matmul(out_ps[:, ql_cur:512],
                                         vt[kc][:, h * 128:(h + 1) * 128],
                                         e_cur[:, ql_cur:512],
                                         start=(kc == klo), stop=(kc == khi),
                                         skip_group_check=True)
                        nc.tensor.matmul(sums_ps[:, ql_cur:512], ones_col[:],
                                         e_cur[:, ql_cur:512],
                                         start=(kc == klo), stop=(kc == khi),
                                         skip_group_check=True)
                        e_cur, ql_cur = e_next, ql_next
                    r_sb = sb.tile([1, 512], F32, tag="r", bufs=2,
                                   name=f"r{qb}_{h}")
                    nc.vector.reciprocal(r_sb[:], sums_ps[:])
                    rb_sb = sb.tile([128, 512], F32, tag="rbs", bufs=2,
                                    name=f"rbs{qb}_{h}")
                    nc.gpsimd.partition_broadcast(rb_sb[:], r_sb[:])
                    a_sb = sb.tile([128, 512], BF16, tag="a", bufs=2,
                                   name=f"a{qb}_{h}")
                    nc.vector.tensor_tensor(out=a_sb[:], in0=out_ps[:],
                                            in1=rb_sb[:],
                                            op=mybir.AluOpType.mult)
                    nc.sync.dma_start(
                        cc_in[qb][h * 128:(h + 1) * 128, :], a_sb[:])
                # one AllGather per block, fired AFTER the last head: a
                # collective in flight during attention stalls the exp stream
                # (shared-semaphore artifact), and the ring is latency-bound
                # (~40us regardless of size) so splitting it never pays
                nc.gpsimd.collective_compute(
                    "AllGather",
                    mybir.AluOpType.bypass,
                    replica_groups=[list(range(N_CORES))],
                    ins=[cc_in[qb][:].opt()],
                    outs=[cc_out[qb][:].opt()],
                )

            def p4_run(sc, o_ps, sbks, suffix="", tail=False):
                # mid-pipeline at-loads go on the gpsimd queue so a pending
                # AllGather never blocks the QKV weight streams; tail at-loads
                # alternate sync/scalar (idle there) because one queue's
                # descriptor-issue rate cannot feed 2-matmul chunks
                for dc in range(N_DC):
                    at_sb = sb.tile([128, 512], BF16, tag="at", bufs=8,
                                    name=f"at{sc}_{dc}{suffix}")
                    if tail:
                        aeng = nc.sync if dc % 2 == 0 else nc.scalar
                    else:
                        aeng = nc.gpsimd
                    aeng.dma_start(at_sb[:],
                                   cc_out[sc][dc * 128:dc * 128 + 128, :])
                    for i, sbk in enumerate(sbks):
                        nc.tensor.matmul(o_ps[i][:],
                                         at_sb[:, sbk * 128:(sbk + 1) * 128],
                                         wo_sb[dc][:], start=(dc == 0),
                                         stop=(dc == N_DC - 1))

            def p4_evict(sc, o_ps, sbks):
                s0 = sc * 512
                for i, sbk in enumerate(sbks):
                    o_sb = sb.tile([128, FC], F32, tag="osb", bufs=2,
                                   name=f"osb{sc}_{sbk}")
                    nc.vector.tensor_copy(o_sb[:], o_ps[i][:])
                    r0 = s0 + sbk * 128
                    nc.scalar.dma_start(out_t[r0:r0 + 128, :], o_sb[:])

            def p4_phase(sc):
                o_ps = [ps.tile([128, 512], F32, tag="acc", bufs=4,
                                name=f"ops{sc}_{i}") for i in range(4)]
                p4_run(sc, o_ps, range(4))
                p4_evict(sc, o_ps, range(4))

            # pipelined schedule; P4(2) is placed after attn(3) so it hides
            # the last AllGather's flight before P4(3) consumes it
            last = N_SC - 1
            kmax_of = (lambda qb: 4 * qb + 3) if causal else (lambda qb: N_KC - 1)
            if not causal:
                # a general mask needs K/V of every block before any
                # attention, so QKV cannot be pipelined with it
                for sc in range(N_SC):
                    qkv_phase(sc)
            for sc in range(N_SC):
                if causal:
                    qkv_phase(sc)
                attn_partial(sc, 0, kmax_of(sc))
                if sc == 1:
                    # only half of P4(0)/P4(1) run mid-pipeline; the other
                    # halves run after attn(3) so ~65us of tensor work covers
                    # the last AllGather's ring flight (which varies 35-140us
                    # with fabric load) before P4(3) needs its data
                    o0a = [ps.tile([128, 512], F32, tag="acc", bufs=4,
                                   name=f"ops0a_{i}") for i in range(2)]
                    p4_run(0, o0a, (0, 1), suffix="a")
                    p4_evict(0, o0a, (0, 1))
                elif sc == 2:
                    o1a = [ps.tile([128, 512], F32, tag="acc", bufs=4,
                                   name=f"ops1a_{i}") for i in range(2)]
                    p4_run(1, o1a, (0, 1), suffix="a")
                    p4_evict(1, o1a, (0, 1))
            o0b = [ps.tile([128, 512], F32, tag="sums", bufs=2,
                           name=f"ops0b_{i}") for i in range(2)]
            p4_run(0, o0b, (2, 3), suffix="b", tail=True)
            p4_evict(0, o0b, (2, 3))
            o1b = [ps.tile([128, 512], F32, tag="st", bufs=2,
                           name=f"ops1b_{i}") for i in range(2)]
            p4_run(1, o1b, (2, 3), suffix="b", tail=True)
            p4_evict(1, o1b, (2, 3))
            o2t = [ps.tile([128, 512], F32, tag="acc", bufs=4,
                           name=f"ops{last-1}_{i}") for i in range(4)]
            p4_run(last - 1, o2t, range(4), tail=True)
            p4_evict(last - 1, o2t, range(4))
            o3 = [ps.tile([128, 512], F32, tag=t, bufs=2,
                          name=f"ops{last}_{i}")
                  for i, t in enumerate(("st", "st", "sums", "sums"))]
            p4_run(last, o3, range(4), tail=True)
            p4_evict(last, o3, range(4))

    nc.finalize()
    return nc


_MODULE_CACHE: dict = {}


def _get_module(causal: bool):
    if causal not in _MODULE_CACHE:
        _MODULE_CACHE[causal] = _build_module(causal)
    return _MODULE_CACHE[causal]


def _rope_perm() -> np.ndarray:
    """Per-head permutation: interleaved pairs (2j, 2j+1) -> (j, j+64)."""
    p = np.empty(HD, dtype=np.int64)
    p[0:64] = np.arange(0, HD, 2)
    p[64:128] = np.arange(1, HD, 2)
    full = np.concatenate([h * HD + p for h in range(HPC)])
    return full


def _canonical_causal_mask() -> np.ndarray:
    m = np.where(np.tril(np.ones((S, S), dtype=bool)), np.float32(0.0),
                 np.float32(NEG))
    return m.astype(np.float32)


def _numpy_fallback(x, freqs_cos, freqs_sin, mask, input_indexes, cache_k,
                    cache_v, wq, wk, wv, wo):
    """Exact reference reimplementation (host, fp32). Only used for inputs
    the device kernel does not model (non-arange cache indexes)."""
    B = x.shape[0]
    xf = x.astype(np.float32)

    def rope(t):
        tr = t[..., 0::2]
        ti = t[..., 1::2]
        c = freqs_cos[None, :, None, :]
        s = freqs_sin[None, :, None, :]
        outr = tr * c - ti * s
        outi = tr * s + ti * c
        return np.stack([outr, outi], axis=-1).reshape(t.shape)

    xq = (xf @ wq.T).reshape(B, S, N_HEADS, HD)
    xk = (xf @ wk.T).reshape(B, S, N_HEADS, HD)
    xv = (xf @ wv.T).reshape(B, S, N_HEADS, HD)
    xq = rope(xq)
    xk = rope(xk)
    keys = np.array(cache_k)
    vals = np.array(cache_v)
    keys[:, input_indexes] = xk
    vals[:, input_indexes] = xv
    scores = np.einsum("bqhd,bkhd->bhqk", xq, keys) / np.sqrt(HD)
    scores = scores + mask
    scores = scores - scores.max(axis=-1, keepdims=True)
    e = np.exp(scores)
    probs = e / e.sum(axis=-1, keepdims=True)
    out = np.einsum("bhqk,bkhd->bqhd", probs, vals)
    return (out.reshape(B, S, N_HEADS * HD) @ wo.T).astype(np.float32)


def _bf16(a: np.ndarray):
    import ml_dtypes
    return np.ascontiguousarray(a.astype(ml_dtypes.bfloat16))


def _prepare_in_maps(x, freqs_cos, freqs_sin, mask, wq, wk, wv, wo, causal):
    x2 = np.ascontiguousarray(x.reshape(S, D), dtype=np.float32)
    xTf = x2.T  # [D, S]
    xT = _bf16(np.concatenate(
        [xTf[:, sc * 512:(sc + 1) * 512] for sc in range(N_SC)], axis=0))

    cosb = np.empty((128, S), dtype=np.float32)
    sinb = np.empty((128, S), dtype=np.float32)
    fc = np.asarray(freqs_cos, dtype=np.float32).T  # [64, S]
    fs = np.asarray(freqs_sin, dtype=np.float32).T
    cosb[0:64] = fc
    cosb[64:128] = fc
    sinb[0:64] = fs
    sinb[64:128] = fs

    perm = _rope_perm()

    if causal:
        kl = np.arange(128, dtype=np.int64)[:, None]
        ql = np.arange(512, dtype=np.int64)[None, :]
        bmask = np.concatenate(
            [(kl <= ql - 128 * j).astype(np.float32) for j in range(4)], axis=1)
        bmask = _bf16(bmask)
    else:
        maskT = np.ascontiguousarray(
            (np.asarray(mask, dtype=np.float32)[0, 0].T) / np.float32(SCALE))

    in_maps = []
    for c in range(N_CORES):
        r0 = c * FC
        wq_c = np.asarray(wq[r0:r0 + FC], dtype=np.float32)[perm]
        wk_c = np.asarray(wk[r0:r0 + FC], dtype=np.float32)[perm]
        wqk_c = np.concatenate([wq_c, wk_c], axis=0)      # [1024, D]
        wv_c = np.asarray(wv[r0:r0 + FC], dtype=np.float32)
        wo_c = np.asarray(wo[r0:r0 + FC], dtype=np.float32)
        m = {
            "xT": xT,
            "wqk_t": _bf16(wqk_c.T),
            "wv_t": _bf16(wv_c.T),
            "wo_t": _bf16(wo_c.T),
            "cosb": cosb,
            "sinb": sinb,
        }
        if causal:
            m["bmask"] = bmask
        else:
            m["maskT"] = maskT
        in_maps.append(m)
    return in_maps


def _run(inputs: dict, trace: bool = False):
    x = np.asarray(inputs["x"])
    freqs_cos = np.asarray(inputs["freqs_cos"])
    freqs_sin = np.asarray(inputs["freqs_sin"])
    mask = np.asarray(inputs["mask"], dtype=np.float32)
    input_indexes = np.asarray(inputs["input_indexes"])
    wq = np.asarray(inputs["wq"])
    wk = np.asarray(inputs["wk"])
    wv = np.asarray(inputs["wv"])
    wo = np.asarray(inputs["wo"])

    if not np.array_equal(input_indexes.astype(np.int64), np.arange(S)):
        out = _numpy_fallback(x, freqs_cos, freqs_sin, mask, input_indexes,
                              inputs["cache_k"], inputs["cache_v"], wq, wk, wv, wo)
        return out, None

    causal = np.array_equal(mask[0, 0], _canonical_causal_mask())
    nc = _get_module(causal)
    in_maps = _prepare_in_maps(x, freqs_cos, freqs_sin, mask, wq, wk, wv, wo,
                               causal)
    res = run_bass_kernel_spmd(nc, in_maps, core_ids=list(range(N_CORES)),
                               trace=trace)
    out = np.concatenate([res.results[c]["out"] for c in range(N_CORES)],
                         axis=1)
    return out.reshape(1, S, D).astype(np.float32), res


def kernel(**inputs) -> np.ndarray:
    out, _ = _run(inputs, trace=False)
    return out


# revision 53
# speedup vs baseline: 1.0604x; 1.0130x over previous
"""Trainium2 Bass kernel for a 32-head causal attention layer.

Problem: B=1, S=2048, D=4096, 32 heads x 128 head-dim, fp32, llama-style
interleaved RoPE on q/k, KV-cache index_copy (identity for arange indexes),
additive mask + softmax, output projection.

Sharding (8 NeuronCores, tensor-parallel by heads):
  - core c owns heads [4c, 4c+4): wq/wk/wv output rows [512c, 512c+512)
  - per-core: QKV projections -> RoPE -> attention -> attn.T shard [512, 2048]
  - 4 chunked on-device AllGathers (one per 512-query block)
  - core c computes output column shard out[:, 512c:512c+512] = attn @ wo_c.T
  - host concatenates the 8 column shards (pure unshard, no arithmetic)

Pipelined schedule: causal attention for query block qb only needs K/V of
seq chunks <= qb, so the program interleaves
  QKV(0), attn(0)+AG(0), QKV(1), attn(1)+AG(1), P4(0a), QKV(2),
  attn(2)+AG(2), P4(1a), QKV(3), attn(3)+AG(3), P4(0b), P4(1b), P4(2), P4(3)
which starts the collectives ~4x earlier and keeps the tensor engine
continuously busy (holding its p-state near max clock).  P4(0)/P4(1) are
split in seq halves so ~65us of output-projection matmuls sit between the
last AllGather's launch and P4(3)'s first use of its data — the ring
flight varies 35-140us with fabric load and is hidden up to ~70us.
Collectives never overlap attention compute (an in-flight collective
stalls the exp stream via a shared-semaphore artifact), and each AG fires
once per block: the ring is latency-bound (~40us regardless of payload) so
splitting a collective never pays.  Tail P4 at-loads alternate sync/scalar
because one queue's descriptor-issue rate cannot feed 2-matmul chunks;
mid-pipeline at-loads ride gpsimd so a pending AllGather never delays the
QKV weight streams.

All matmul operands are bf16 (1 cycle/row on the PE, half the DMA bytes of
fp32r); PSUM accumulation stays fp32.  End-to-end max-abs error vs the fp32
reference is ~3e-3 of the output max (budget 2e-2).

RoPE trick: weight rows of wq/wk are permuted per head on the host so the
interleaved pairs (2j, 2j+1) become (j, j+64).  Scores are invariant under
a per-head orthogonal permutation applied to both q and k, and the rotation
then only needs partition-range [0:64]/[64:128] cross-multiplies, which map
to plain DVE tensor_tensor ops (no strided partition access).

The 1/sqrt(128) score scale is folded into the Exp activation's scale
operand.  Softmax runs over the partition (key) axis: scores are computed
transposed st[k, q] = K Q^T, summed with a ones-vector matmul, and
normalized after the PV matmul via a reciprocal + outer-product broadcast.
"""

import numpy as np

import concourse.bass as bass
import concourse.mybir as mybir
import concourse.tile as tile
from concourse import bacc
from concourse.bass_utils import run_bass_kernel_spmd

F32 = mybir.dt.float32
BF16 = mybir.dt.bfloat16

S = 2048
D = 4096
HD = 128
N_HEADS = 32
N_CORES = 8
HPC = N_HEADS // N_CORES          # heads per core = 4
FC = HPC * HD                     # features per core = 512
N_DC = D // 128                   # 32 contraction chunks
N_SC = S // 512                   # 4 seq chunks of 512
N_KC = S // 128                   # 16 key chunks of 128
SCALE = 1.0 / np.sqrt(HD)
NEG = -1e9


def _build_module(causal: bool):
    nc = bacc.Bacc(num_devices=N_CORES)

    xT = nc.dram_tensor("xT", [N_SC * D, 512], BF16, kind="ExternalInput")
    wqk_t = nc.dram_tensor("wqk_t", [D, 2 * FC], BF16, kind="ExternalInput")
    wv_t = nc.dram_tensor("wv_t", [D, FC], BF16, kind="ExternalInput")
    wo_t = nc.dram_tensor("wo_t", [D, FC], BF16, kind="ExternalInput")
    cosb = nc.dram_tensor("cosb", [128, S], F32, kind="ExternalInput")
    sinb = nc.dram_tensor("sinb", [128, S], F32, kind="ExternalInput")
    if causal:
        bmask = nc.dram_tensor("bmask", [128, 4 * 512], BF16, kind="ExternalInput")
    else:
        maskT = nc.dram_tensor("maskT", [S, S], F32, kind="ExternalInput")
    out_t = nc.dram_tensor("out", [S, FC], F32, kind="ExternalOutput")

    with tile.TileContext(nc) as tc:
        with tc.tile_pool(name="const", bufs=1) as constp, \
             tc.tile_pool(name="dram", bufs=1, space="DRAM") as dram, \
             tc.tile_pool(name="sb", bufs=1) as sb, \
             tc.tile_pool(name="ps", bufs=1, space="PSUM") as ps:
            cc_in = [dram.tile([FC, 512], BF16, name=f"cc_in{i}")
                     for i in range(N_SC)]
            cc_out = [dram.tile([D, 512], BF16, addr_space="Shared",
                                name=f"cc_out{i}") for i in range(N_SC)]

            ones_f = constp.tile([128, 1], F32, tag="ones_f")
            nc.vector.memset(ones_f[:], 1.0)
            ones_col = constp.tile([128, 1], BF16, tag="ones_col")
            nc.vector.tensor_copy(ones_col[:], ones_f[:])



            cos_sb = constp.tile([128, S], F32, tag="cos")
            sin_sb = constp.tile([128, S], F32, tag="sin")
            nc.gpsimd.dma_start(cos_sb[:], cosb[:])
            nc.gpsimd.dma_start(sin_sb[:], sinb[:])
            if causal:
                bm_sb = constp.tile([128, 4 * 512], BF16, tag="bm")
                nc.gpsimd.dma_start(bm_sb[:], bmask[:])

            # persistent q/k/v activation tiles (bf16)
            qt = [constp.tile([128, S], BF16, tag=f"qt{h}", name=f"qt{h}")
                  for h in range(HPC)]
            kt = [constp.tile([128, S], BF16, tag=f"kt{h}", name=f"kt{h}")
                  for h in range(HPC)]
            vt = [constp.tile([128, FC], BF16, tag=f"vt{b}", name=f"vt{b}")
                  for b in range(N_KC)]

            # resident wo tiles: on the gpsimd queue so they do not delay the
            # QKV weight/activation streams (sync+scalar queues)
            wo_sb = [constp.tile([128, FC], BF16, tag=f"wo{dc}", name=f"wo{dc}")
                     for dc in range(N_DC)]
            for dc in range(N_DC):
                nc.gpsimd.dma_start(wo_sb[dc][:], wo_t[dc * 128:(dc + 1) * 128, :])

            def rope_evict(acc, dst, sc):
                s0 = sc * 512
                cs = cos_sb[:, s0:s0 + 512]
                sn = sin_sb[:, s0:s0 + 512]
                snap = sb.tile([128, 512], F32, tag="snap", bufs=1)
                t14 = sb.tile([128, 512], F32, tag="t1", bufs=2)
                t2 = sb.tile([128, 512], F32, tag="t2", bufs=2)
                t3 = sb.tile([128, 512], F32, tag="t3", bufs=2)
                mul = mybir.AluOpType.mult
                # single full-width copy is the ONLY acc read, so the PSUM
                # bank frees for the next sweep's matmuls after ~0.5us
                # instead of three dependent multiplies (~2us)
                nc.vector.tensor_copy(snap[:], acc[:])
                nc.vector.tensor_tensor(out=t14[:], in0=snap[:], in1=cs[:],
                                        op=mul)
                # sin_sb holds the sin table duplicated in both halves, so
                # the operand can be sourced at the base partition matching
                # snap (both-SBUF tensor_tensor requires equal bases)
                nc.vector.tensor_tensor(out=t2[0:64, :], in0=snap[64:128, :],
                                        in1=sn[64:128, :], op=mul)
                nc.vector.tensor_tensor(out=t3[64:128, :], in0=snap[0:64, :],
                                        in1=sn[0:64, :], op=mul)
                nc.vector.tensor_tensor(out=dst[0:64, s0:s0 + 512],
                                        in0=t14[0:64, :], in1=t2[0:64, :],
                                        op=mybir.AluOpType.subtract)
                nc.vector.tensor_tensor(out=dst[64:128, s0:s0 + 512],
                                        in0=t14[64:128, :], in1=t3[64:128, :],
                                        op=mybir.AluOpType.add)

            def qkv_phase(sc):
                s0 = sc * 512
                # q sweep (4 PSUM accumulators), then k sweep, then v sweep.
                xts = []
                for which in ("q", "k"):
                    off = 0 if which == "q" else FC
                    accs = [ps.tile([128, 512], F32, tag="acc", bufs=4,
                                    name=f"{which}ps{sc}_{h}") for h in range(HPC)]
                    for dc in range(N_DC):
                        d0 = dc * 128
                        if which == "q":
                            xt_sb = sb.tile([128, 512], BF16, tag="xt", bufs=40,
                                            name=f"xt{sc}_{dc}")
                            nc.sync.dma_start(
                                xt_sb[:], xT[sc * D + d0:sc * D + d0 + 128, :])
                            xts.append(xt_sb)
                        w_sb = sb.tile([128, FC], BF16, tag="wqk", bufs=12,
                                       name=f"w{which}{sc}_{dc}")
                        if which == "q":
                            weng = nc.scalar  # sync is busy with the xt loads
                        else:
                            weng = nc.scalar if dc % 2 == 0 else nc.sync
                        weng.dma_start(w_sb[:], wqk_t[d0:d0 + 128, off:off + FC])
                        for h in range(HPC):
                            f0 = h * 128
                            nc.tensor.matmul(accs[h][:], w_sb[:, f0:f0 + 128],
                                             xts[dc][:], start=(dc == 0),
                                             stop=(dc == N_DC - 1))
                    dst = qt if which == "q" else kt
                    for h in range(HPC):
                        rope_evict(accs[h], dst[h], sc)

                v_ps = [ps.tile([128, 512], F32, tag="acc", bufs=4,
                                name=f"vps{sc}_{i}") for i in range(4)]
                for dc in range(N_DC):
                    d0 = dc * 128
                    wv_sb = sb.tile([128, FC], BF16, tag="wv", bufs=12,
                                    name=f"wv{sc}_{dc}")
                    veng = nc.scalar if dc % 2 == 0 else nc.sync
                    veng.dma_start(wv_sb[:], wv_t[d0:d0 + 128, :])
                    for sbk in range(4):
                        nc.tensor.matmul(v_ps[sbk][:],
                                         xts[dc][:, sbk * 128:(sbk + 1) * 128],
                                         wv_sb[:], start=(dc == 0),
                                         stop=(dc == N_DC - 1))
                for sbk in range(4):
                    nc.vector.tensor_copy(vt[sc * 4 + sbk][:], v_ps[sbk][:])

            def attn_partial(qb, klo, khi):
                """Attention for query block qb over key chunks [klo, khi],
                normalizing and AllGathering the result."""
                q0 = qb * 512
                for h in range(HPC):
                    out_ps = ps.tile([128, 512], F32, tag="acc", bufs=4,
                                     name=f"o{qb}_{h}_{klo}")
                    sums_ps = ps.tile([1, 512], F32, tag="sums", bufs=2,
                                      name=f"s{qb}_{h}_{klo}")

                    def issue_st(kc):
                        k0 = kc * 128
                        # diagonal chunk j masks query columns < j*128 and
                        # could be narrowed, but per-matmul fixed overhead
                        # (~160ns) eats the saved rows: a 384-wide matmul
                        # measures 300ns vs 293ns for 512-wide, so keep full
                        # width
                        j = kc - 4 * qb if causal else -1
                        qlo = 0
                        st_ps = ps.tile([128, 512], F32, tag="st", bufs=2,
                                        name=f"st{qb}_{h}_{kc}")
                        nc.tensor.matmul(st_ps[:, qlo:512],
                                         kt[h][:, k0:k0 + 128],
                                         qt[h][:, q0 + qlo:q0 + 512],
                                         start=True, stop=True)
                        e_sb = sb.tile([128, 512], BF16, tag="e", bufs=6,
                                       name=f"e{qb}_{h}_{kc}")
                        if causal:
                            nc.scalar.activation(
                                e_sb[:, qlo:512], st_ps[:, qlo:512],
                                mybir.ActivationFunctionType.Exp,
                                scale=float(SCALE))
                            if 0 <= j <= 3:
                                nc.vector.tensor_tensor(
                                    out=e_sb[:, qlo:512],
                                    in0=e_sb[:, qlo:512],
                                    in1=bm_sb[:, j * 512 + qlo:(j + 1) * 512],
                                    op=mybir.AluOpType.mult)
                        else:
                            mt_sb = sb.tile([128, 512], F32, tag="mt", bufs=2,
                                            name=f"mt{qb}_{h}_{kc}")
                            nc.sync.dma_start(
                                mt_sb[:], maskT[k0:k0 + 128, q0:q0 + 512])
                            nc.vector.tensor_tensor(
                                out=st_ps[:], in0=st_ps[:], in1=mt_sb[:],
                                op=mybir.AluOpType.add)
                            nc.scalar.activation(
                                e_sb[:], st_ps[:],
                                mybir.ActivationFunctionType.Exp,
                                scale=float(SCALE))
                        return e_sb, qlo

                    e_cur, ql_cur = issue_st(klo)
                    for kc in range(klo, khi + 1):
                        e_next, ql_next = (issue_st(kc + 1) if kc < khi
                                           else (None, 0))
                        nc.tensor.matmul(out_ps[:, ql_cur:512],
                                         vt[kc][:, h * 128:(h + 1) * 128],
                                         e_cur[:, ql_cur:512],
                                         start=(kc == klo), stop=(kc == khi),
                                         skip_group_check=True)
                        nc.tensor.matmul(sums_ps[:, ql_cur:512], ones_col[:],
                                         e_cur[:, ql_cur:512],
                                         start=(kc == klo), stop=(kc == khi),
                                         skip_group_check=True)
                        e_cur, ql_cur = e_next, ql_next
                    r_sb = sb.tile([1, 512], F32, tag="r", bufs=2,
                                   name=f"r{qb}_{h}")
                    nc.vector.reciprocal(r_sb[:], sums_ps[:])
                    rb_sb = sb.tile([128, 512], F32, tag="rbs", bufs=2,
                                    name=f"rbs{qb}_{h}")
                    nc.gpsimd.partition_broadcast(rb_sb[:], r_sb[:])
                    a_sb = sb.tile([128, 512], BF16, tag="a", bufs=2,
                                   name=f"a{qb}_{h}")
                    nc.vector.tensor_tensor(out=a_sb[:], in0=out_ps[:],
                                            in1=rb_sb[:],
                                            op=mybir.AluOpType.mult)
                    nc.sync.dma_start(
                        cc_in[qb][h * 128:(h + 1) * 128, :], a_sb[:])
                # one AllGather per block, fired AFTER the last head: a
                # collective in flight during attention stalls the exp stream
                # (shared-semaphore artifact), and the ring is latency-bound
                # (~40us regardless of size) so splitting it never pays
                nc.gpsimd.collective_compute(
                    "AllGather",
                    mybir.AluOpType.bypass,
                    replica_groups=[list(range(N_CORES))],
                    ins=[cc_in[qb][:].opt()],
                    outs=[cc_out[qb][:].opt()],
                )

            def p4_run(sc, o_ps, sbks, suffix="", tail=False):
                # mid-pipeline at-loads go on the gpsimd queue so a pending
                # AllGather never blocks the QKV weight streams; tail at-loads
                # alternate sync/scalar (idle there) because one queue's
                # descriptor-issue rate cannot feed 2-matmul chunks
                for dc in range(N_DC):
                    at_sb = sb.tile([128, 512], BF16, tag="at", bufs=8,
                                    name=f"at{sc}_{dc}{suffix}")
                    if tail:
                        aeng = nc.sync if dc % 2 == 0 else nc.scalar
                    else:
                        aeng = nc.gpsimd
                    aeng.dma_start(at_sb[:],
                                   cc_out[sc][dc * 128:dc * 128 + 128, :])
                    for i, sbk in enumerate(sbks):
                        nc.tensor.matmul(o_ps[i][:],
                                         at_sb[:, sbk * 128:(sbk + 1) * 128],
                                         wo_sb[dc][:], start=(dc == 0),
                                         stop=(dc == N_DC - 1))

            def p4_evict(sc, o_ps, sbks):
                s0 = sc * 512
                for i, sbk in enumerate(sbks):
                    o_sb = sb.tile([128, FC], F32, tag="osb", bufs=2,
                                   name=f"osb{sc}_{sbk}")
                    nc.vector.tensor_copy(o_sb[:], o_ps[i][:])
                    r0 = s0 + sbk * 128
                    nc.scalar.dma_start(out_t[r0:r0 + 128, :], o_sb[:])

            def p4_phase(sc):
                o_ps = [ps.tile([128, 512], F32, tag="acc", bufs=4,
                                name=f"ops{sc}_{i}") for i in range(4)]
                p4_run(sc, o_ps, range(4))
                p4_evict(sc, o_ps, range(4))

            # pipelined schedule; P4(2) is placed after attn(3) so it hides
            # the last AllGather's flight before P4(3) consumes it
            last = N_SC - 1
            kmax_of = (lambda qb: 4 * qb + 3) if causal else (lambda qb: N_KC - 1)
            if not causal:
                # a general mask needs K/V of every block before any
                # attention, so QKV cannot be pipelined with it
                for sc in range(N_SC):
                    qkv_phase(sc)
            for sc in range(N_SC):
                if causal:
                    qkv_phase(sc)
                attn_partial(sc, 0, kmax_of(sc))
                if sc == 1:
                    # only half of P4(0)/P4(1) run mid-pipeline; the other
                    # halves run after attn(3) so ~65us of tensor work covers
                    # the last AllGather's ring flight (which varies 35-140us
                    # with fabric load) before P4(3) needs its data
                    o0a = [ps.tile([128, 512], F32, tag="acc", bufs=4,
                                   name=f"ops0a_{i}") for i in range(2)]
                    p4_run(0, o0a, (0, 1), suffix="a")
                    p4_evict(0, o0a, (0, 1))
                elif sc == 2:
                    o1a = [ps.tile([128, 512], F32, tag="acc", bufs=4,
                                   name=f"ops1a_{i}") for i in range(2)]
                    p4_run(1, o1a, (0, 1), suffix="a")
                    p4_evict(1, o1a, (0, 1))
            o0b = [ps.tile([128, 512], F32, tag="sums", bufs=2,
                           name=f"ops0b_{i}") for i in range(2)]
            p4_run(0, o0b, (2, 3), suffix="b", tail=True)
            p4_evict(0, o0b, (2, 3))
            o1b = [ps.tile([128, 512], F32, tag="st", bufs=2,
                           name=f"ops1b_{i}") for i in range(2)]
            p4_run(1, o1b, (2, 3), suffix="b", tail=True)
            p4_evict(1, o1b, (2, 3))
            o2t = [ps.tile([128, 512], F32, tag="acc", bufs=4,
                           name=f"ops{last-1}_{i}") for i in range(4)]
            p4_run(last - 1, o2t, range(4), tail=True)
            p4_evict(last - 1, o2t, range(4))
            o3 = [ps.tile([128, 512], F32, tag=t, bufs=2,
                          name=f"ops{last}_{i}")
                  for i, t in enumerate(("st", "st", "sums", "sums"))]
            p4_run(last, o3, range(4), tail=True)
            p4_evict(last, o3, range(4))

    nc.finalize()
    return nc


_MODULE_CACHE: dict = {}


def _get_module(causal: bool):
    if causal not in _MODULE_CACHE:
        _MODULE_CACHE[causal] = _build_module(causal)
    return _MODULE_CACHE[causal]


def _rope_perm() -> np.ndarray:
    """Per-head permutation: interleaved pairs (2j, 2j+1) -> (j, j+64)."""
    p = np.empty(HD, dtype=np.int64)
    p[0:64] = np.arange(0, HD, 2)
    p[64:128] = np.arange(1, HD, 2)
    full = np.concatenate([h * HD + p for h in range(HPC)])
    return full


def _canonical_causal_mask() -> np.ndarray:
    m = np.where(np.tril(np.ones((S, S), dtype=bool)), np.float32(0.0),
                 np.float32(NEG))
    return m.astype(np.float32)


def _numpy_fallback(x, freqs_cos, freqs_sin, mask, input_indexes, cache_k,
                    cache_v, wq, wk, wv, wo):
    """Exact reference reimplementation (host, fp32). Only used for inputs
    the device kernel does not model (non-arange cache indexes)."""
    B = x.shape[0]
    xf = x.astype(np.float32)

    def rope(t):
        tr = t[..., 0::2]
        ti = t[..., 1::2]
        c = freqs_cos[None, :, None, :]
        s = freqs_sin[None, :, None, :]
        outr = tr * c - ti * s
        outi = tr * s + ti * c
        return np.stack([outr, outi], axis=-1).reshape(t.shape)

    xq = (xf @ wq.T).reshape(B, S, N_HEADS, HD)
    xk = (xf @ wk.T).reshape(B, S, N_HEADS, HD)
    xv = (xf @ wv.T).reshape(B, S, N_HEADS, HD)
    xq = rope(xq)
    xk = rope(xk)
    keys = np.array(cache_k)
    vals = np.array(cache_v)
    keys[:, input_indexes] = xk
    vals[:, input_indexes] = xv
    scores = np.einsum("bqhd,bkhd->bhqk", xq, keys) / np.sqrt(HD)
    scores = scores + mask
    scores = scores - scores.max(axis=-1, keepdims=True)
    e = np.exp(scores)
    probs = e / e.sum(axis=-1, keepdims=True)
    out = np.einsum("bhqk,bkhd->bqhd", probs, vals)
    return (out.reshape(B, S, N_HEADS * HD) @ wo.T).astype(np.float32)


def _bf16(a: np.ndarray):
    import ml_dtypes
    return np.ascontiguousarray(a.astype(ml_dtypes.bfloat16))


def _prepare_in_maps(x, freqs_cos, freqs_sin, mask, wq, wk, wv, wo, causal):
    x2 = np.ascontiguousarray(x.reshape(S, D), dtype=np.float32)
    xTf = x2.T  # [D, S]
    xT = _bf16(np.concatenate(
        [xTf[:, sc * 512:(sc + 1) * 512] for sc in range(N_SC)], axis=0))

    cosb = np.empty((128, S), dtype=np.float32)
    sinb = np.empty((128, S), dtype=np.float32)
    fc = np.asarray(freqs_cos, dtype=np.float32).T  # [64, S]
    fs = np.asarray(freqs_sin, dtype=np.float32).T
    cosb[0:64] = fc
    cosb[64:128] = fc
    sinb[0:64] = fs
    sinb[64:128] = fs

    perm = _rope_perm()

    if causal:
        kl = np.arange(128, dtype=np.int64)[:, None]
        ql = np.arange(512, dtype=np.int64)[None, :]
        bmask = np.concatenate(
            [(kl <= ql - 128 * j).astype(np.float32) for j in range(4)], axis=1)
        bmask = _bf16(bmask)
    else:
        maskT = np.ascontiguousarray(
            (np.asarray(mask, dtype=np.float32)[0, 0].T) / np.float32(SCALE))

    in_maps = []
    for c in range(N_CORES):
        r0 = c * FC
        wq_c = np.asarray(wq[r0:r0 + FC], dtype=np.float32)[perm]
        wk_c = np.asarray(wk[r0:r0 + FC], dtype=np.float32)[perm]
        wqk_c = np.concatenate([wq_c, wk_c], axis=0)      # [1024, D]
        wv_c = np.asarray(wv[r0:r0 + FC], dtype=np.float32)
        wo_c = np.asarray(wo[r0:r0 + FC], dtype=np.float32)
        m = {
            "xT": xT,
            "wqk_t": _bf16(wqk_c.T),
            "wv_t": _bf16(wv_c.T),
            "wo_t": _bf16(wo_c.T),
            "cosb": cosb,
            "sinb": sinb,
        }
        if causal:
            m["bmask"] = bmask
        else:
            m["maskT"] = maskT
        in_maps.append(m)
    return in_maps


def _run(inputs: dict, trace: bool = False):
    x = np.asarray(inputs["x"])
    freqs_cos = np.asarray(inputs["freqs_cos"])
    freqs_sin = np.asarray(inputs["freqs_sin"])
    mask = np.asarray(inputs["mask"], dtype=np.float32)
    input_indexes = np.asarray(inputs["input_indexes"])
    wq = np.asarray(inputs["wq"])
    wk = np.asarray(inputs["wk"])
    wv = np.asarray(inputs["wv"])
    wo = np.asarray(inputs["wo"])

    if not np.array_equal(input_indexes.astype(np.int64), np.arange(S)):
        out = _numpy_fallback(x, freqs_cos, freqs_sin, mask, input_indexes,
                              inputs["cache_k"], inputs["cache_v"], wq, wk, wv, wo)
        return out, None

    causal = np.array_equal(mask[0, 0], _canonical_causal_mask())
    nc = _get_module(causal)
    in_maps = _prepare_in_maps(x, freqs_cos, freqs_sin, mask, wq, wk, wv, wo,
                               causal)
    res = run_bass_kernel_spmd(nc, in_maps, core_ids=list(range(N_CORES)),
                               trace=trace)
    out = np.concatenate([res.results[c]["out"] for c in range(N_CORES)],
                         axis=1)
    return out.reshape(1, S, D).astype(np.float32), res


def kernel(**inputs) -> np.ndarray:
    out, _ = _run(inputs, trace=False)
    return out
